# revision 1
# baseline (speedup 1.0000x reference)
"""Multi-head attention (B=2, S=2048, E=1024, H=16) on 8 Trainium2 cores.

Sharding: core c -> (batch b = c//4, head-group g = c%4 of 4 heads).
Each core computes Q/K/V projections for its 4 heads (256 features),
full attention for those heads, and a partial output projection
(256 rows of Wo). Host sums the 4 partials per batch element and adds bo.

On-chip layouts (per core):
  qt/kt: (128 feat-part, pair, 2048 tok)  "transposed" proj outputs; the
         128 partitions hold two heads (64+64) per pair index.
  v:     (128 tok-part, 16 tok-tiles, 4*65) per head 64 dims + a ones
         column; the ones column makes P@V_aug also produce the softmax
         denominator row.
  scores are computed transposed (key-pos on partitions, query on free)
  so exp runs on ACT along free dim and P tiles feed P@V directly as the
  moving operand; no transposes anywhere in the pipeline.
"""

import numpy as np

B, S, E, H = 2, 2048, 1024, 16
D = 64
NCORES = 8
FPC = 256  # features (head dims) per core = 4 heads

# 'f32' (exact, 4 cyc/row), 'f32r' (fp32 storage, fast PE mode), 'bf16'
MODE = "f32r"

_PROGRAMS = {}
LAST_RESULT = None
TRACE = False
TRACE_DIR = None


def _build(mode):
    import concourse.tile as tile
    from concourse import bacc, mybir

    f32 = mybir.dt.float32
    if mode == "bf16":
        DT = mybir.dt.bfloat16
    else:
        DT = f32

    def c(ap):
        # matmul-operand view: reinterpret fp32 as float32r for the fast PE path
        if mode == "f32r":
            return ap.bitcast(mybir.dt.float32r)
        return ap

    nc = bacc.Bacc("TRN2", target_bir_lowering=False, debug=False,
                   num_devices=NCORES)

    xq_ap = nc.dram_tensor("xq", [E, S], DT, kind="ExternalInput").ap()
    xk_ap = nc.dram_tensor("xk", [E, S], DT, kind="ExternalInput").ap()
    xv_ap = nc.dram_tensor("xv", [E, S], DT, kind="ExternalInput").ap()
    wq_ap = nc.dram_tensor("wq", [E, FPC], DT, kind="ExternalInput").ap()
    wk_ap = nc.dram_tensor("wk", [E, FPC], DT, kind="ExternalInput").ap()
    wv_ap = nc.dram_tensor("wv", [E, FPC], DT, kind="ExternalInput").ap()
    wo_ap = nc.dram_tensor("wo", [FPC, E], DT, kind="ExternalInput").ap()
    bqk_ap = nc.dram_tensor("bqk", [4, 128], f32, kind="ExternalInput").ap()
    bv_ap = nc.dram_tensor("bv", [1, FPC], DT, kind="ExternalInput").ap()
    y_ap = nc.dram_tensor("y", [S, E], f32, kind="ExternalOutput").ap()

    Identity = mybir.ActivationFunctionType.Identity
    Exp = mybir.ActivationFunctionType.Exp

    with tile.TileContext(nc) as tc:
        with tc.tile_pool(name="persist", bufs=1) as persist:
            wq_sb = persist.tile([128, 8, FPC], DT, name="wq_sb")
            wk_sb = persist.tile([128, 8, FPC], DT, name="wk_sb")
            wv_sb = persist.tile([128, 8, FPC], DT, name="wv_sb")
            wo_sb = persist.tile([128, 2, E], DT, name="wo_sb")
            nc.sync.dma_start(wq_sb, wq_ap.rearrange("(k p) m -> p k m", p=128))
            nc.sync.dma_start(wk_sb, wk_ap.rearrange("(k p) m -> p k m", p=128))
            nc.sync.dma_start(wv_sb, wv_ap.rearrange("(k p) m -> p k m", p=128))
            nc.sync.dma_start(wo_sb, wo_ap.rearrange("(k p) m -> p k m", p=128))
            bqk_sb = persist.tile([128, 4], f32, name="bqk_sb")
            nc.sync.dma_start(bqk_sb, bqk_ap.rearrange("m p -> p m"))
            bv_sb = persist.tile([1, FPC], DT, name="bv_sb")
            nc.sync.dma_start(bv_sb, bv_ap)
            ones_sb = persist.tile([1, 128], DT, name="ones_sb")
            nc.vector.memset(ones_sb, 1.0)
            ones32 = persist.tile([1, 64], f32, name="ones32")
            nc.vector.memset(ones32, 1.0)

            qt_sb = persist.tile([128, 2, S], DT, name="qt_sb")
            kt_sb = persist.tile([128, 2, S], DT, name="kt_sb")
            v_sb = persist.tile([128, 16, 4 * 65], DT, name="v_sb")
            at_sb = persist.tile([128, 2, S], DT, name="at_sb")
            # ones column per head (index 64 of each 65-wide block)
            vre = v_sb.rearrange("p t (h e) -> p t h e", e=65)
            nc.vector.memset(vre[:, :, :, 64:65], 1.0)

            with tc.tile_pool(name="xs", bufs=3) as xpool, \
                 tc.tile_pool(name="pjps", bufs=8, space="PSUM") as pjps:
                # ---- Q and K projections: out = (feat-part, tok) ----
                for which, xap, w_sb, out_sb, bcol in (
                    (0, xq_ap, wq_sb, qt_sb, 0),
                    (1, xk_ap, wk_sb, kt_sb, 2),
                ):
                    ps = {}
                    for mt in range(2):
                        for nb in range(4):
                            ps[(mt, nb)] = pjps.tile(
                                [128, 512], f32, tag="pj",
                                name=f"pjq_{which}_{mt}_{nb}")
                    for kt in range(8):
                        xt = xpool.tile([128, S], DT, tag="x",
                                        name=f"x_{which}_{kt}")
                        nc.sync.dma_start(xt, xap[kt * 128:(kt + 1) * 128, :])
                        for mt in range(2):
                            for nb in range(4):
                                nc.tensor.matmul(
                                    ps[(mt, nb)],
                                    c(w_sb[:, kt, mt * 128:(mt + 1) * 128]),
                                    c(xt[:, nb * 512:(nb + 1) * 512]),
                                    start=(kt == 0), stop=(kt == 7))
                    for mt in range(2):
                        for nb in range(4):
                            nc.scalar.activation(
                                out_sb[:, mt, nb * 512:(nb + 1) * 512],
                                ps[(mt, nb)], Identity,
                                bias=bqk_sb[:, bcol + mt:bcol + mt + 1])

                # ---- V projection: out = (tok-part, feat), bias via K=1 mm ----
                for half in range(2):
                    psv = [pjps.tile([128, FPC], f32, tag="pj",
                                     name=f"pjv_{half}_{i}") for i in range(8)]
                    for i in range(8):
                        nc.tensor.matmul(psv[i], c(ones_sb.bitcast(f32)[:, :128]
                                                   if mode == "f32r" else ones_sb),
                                         c(bv_sb.bitcast(f32) if mode == "f32r"
                                           else bv_sb),
                                         start=True, stop=False)
                    for kt in range(8):
                        xt = xpool.tile([128, 1024], DT, tag="x",
                                        name=f"xv_{half}_{kt}")
                        nc.sync.dma_start(
                            xt, xv_ap[kt * 128:(kt + 1) * 128,
                                      half * 1024:(half + 1) * 1024])
                        for i in range(8):
                            nc.tensor.matmul(
                                psv[i],
                                c(xt[:, i * 128:(i + 1) * 128]),
                                c(wv_sb[:, kt, :]),
                                start=False, stop=(kt == 7))
                    for i in range(8):
                        tt = half * 8 + i
                        nc.scalar.copy(
                            vre[:, tt, :, 0:64],
                            psv[i].rearrange("p (h d) -> p h d", d=64))

            # ---- attention + output projection ----
            with tc.tile_pool(name="pt", bufs=6) as ptpool, \
                 tc.tile_pool(name="sm", bufs=2) as smpool, \
                 tc.tile_pool(name="ysb", bufs=3) as ypool, \
                 tc.tile_pool(name="scps", bufs=3, space="PSUM") as scps, \
                 tc.tile_pool(name="pvps", bufs=2, space="PSUM") as pvps, \
                 tc.tile_pool(name="rbps", bufs=1, space="PSUM") as rbps, \
                 tc.tile_pool(name="yps", bufs=2, space="PSUM") as yps:
                for qb in range(4):
                    qsl = slice(qb * 512, (qb + 1) * 512)
                    for p in range(2):
                        pvt = [pvps.tile([65, 512], f32, tag="pv",
                                         name=f"pv_{qb}_{p}_{hh}")
                               for hh in range(2)]
                        for kt in range(16):
                            sc = []
                            for hh in range(2):
                                s_ = scps.tile([128, 512], f32, tag="sc",
                                               name=f"sc_{qb}_{p}_{kt}_{hh}")
                                nc.tensor.matmul(
                                    s_,
                                    c(kt_sb[64 * hh:64 * hh + 64, p,
                                            kt * 128:(kt + 1) * 128]),
                                    c(qt_sb[64 * hh:64 * hh + 64, p, qsl]),
                                    start=True, stop=True)
                                sc.append(s_)
                            for hh in range(2):
                                h = 2 * p + hh
                                ptt = ptpool.tile([128, 512], DT,
                                                  tag=f"pt{hh}",
                                                  name=f"pt_{qb}_{p}_{kt}_{hh}")
                                nc.scalar.activation(ptt, sc[hh], Exp,
                                                     scale=0.125)
                                nc.tensor.matmul(
                                    pvt[hh],
                                    c(v_sb[:, kt, 65 * h:65 * h + 65]),
                                    c(ptt),
                                    start=(kt == 0), stop=(kt == 15))
                        for hh in range(2):
                            recip = smpool.tile([1, 512], f32, tag="recip",
                                                name=f"rc_{qb}_{p}_{hh}")
                            nc.vector.reciprocal(recip, pvt[hh][64:65, :])
                            rb = rbps.tile([64, 512], f32, tag="rb",
                                           name=f"rb_{qb}_{p}_{hh}")
                            nc.tensor.matmul(rb, c(ones32), c(recip),
                                             start=True, stop=True)
                            rbs = smpool.tile([64, 512], f32, tag="rbs",
                                              name=f"rbs_{qb}_{p}_{hh}")
                            nc.scalar.copy(rbs, rb)
                            nc.vector.tensor_mul(
                                at_sb[64 * hh:64 * hh + 64, p, qsl],
                                pvt[hh][0:64, :], rbs)
                    # output projection for this query block's 4 token tiles
                    for mt in range(4 * qb, 4 * qb + 4):
                        yp = [yps.tile([128, 512], f32, tag="y",
                                       name=f"yp_{mt}_{nb}") for nb in range(2)]
                        for nb in range(2):
                            for p2 in range(2):
                                nc.tensor.matmul(
                                    yp[nb],
                                    c(at_sb[:, p2, mt * 128:(mt + 1) * 128]),
                                    c(wo_sb[:, p2, nb * 512:(nb + 1) * 512]),
                                    start=(p2 == 0), stop=(p2 == 1))
                        yo = ypool.tile([128, E], f32, tag="yo",
                                        name=f"yo_{mt}")
                        for nb in range(2):
                            nc.vector.tensor_copy(yo[:, nb * 512:(nb + 1) * 512],
                                                  yp[nb])
                        nc.sync.dma_start(y_ap[mt * 128:(mt + 1) * 128, :], yo)

    nc.compile()
    return nc


def _get_program(mode):
    if mode not in _PROGRAMS:
        _PROGRAMS[mode] = _build(mode)
    return _PROGRAMS[mode]


def kernel(q, k, v, mask, Wq, bq, Wk, bk, Wv, bv, Wo, bo):
    global LAST_RESULT
    from concourse.bass_utils import run_bass_kernel_spmd

    mode = MODE
    nc = _get_program(mode)

    if mode == "bf16":
        import ml_dtypes
        cdt = ml_dtypes.bfloat16
    else:
        cdt = np.float32

    def prep(a, dt=None):
        return np.ascontiguousarray(a.astype(dt or cdt))

    q = np.asarray(q); k = np.asarray(k); v = np.asarray(v)
    Wq = np.asarray(Wq); Wk = np.asarray(Wk); Wv = np.asarray(Wv)
    Wo = np.asarray(Wo)
    bq = np.asarray(bq); bk = np.asarray(bk); bv = np.asarray(bv)
    bo = np.asarray(bo)

    xT = [[prep(q[b].T), prep(k[b].T), prep(v[b].T)] for b in range(B)]

    in_maps = []
    for core in range(NCORES):
        b, g = core // 4, core % 4
        r0 = g * FPC
        in_maps.append({
            "xq": xT[b][0], "xk": xT[b][1], "xv": xT[b][2],
            "wq": prep(Wq[r0:r0 + FPC, :].T),
            "wk": prep(Wk[r0:r0 + FPC, :].T),
            "wv": prep(Wv[r0:r0 + FPC, :].T),
            "wo": prep(Wo[:, r0:r0 + FPC].T),
            "bqk": np.stack([bq[r0:r0 + 128], bq[r0 + 128:r0 + FPC],
                             bk[r0:r0 + 128], bk[r0 + 128:r0 + FPC]]
                            ).astype(np.float32),
            "bv": prep(bv[r0:r0 + FPC][None, :]),
        })

    kwargs = {}
    if TRACE:
        kwargs = {"trace": True, "tmpdir": TRACE_DIR}
    res = run_bass_kernel_spmd(nc, in_maps, list(range(NCORES)), **kwargs)
    LAST_RESULT = res

    y = np.zeros((B, S, E), np.float32)
    for core in range(NCORES):
        y[core // 4] += res.results[core]["y"]
    y += bo.astype(np.float32)
    return y


# revision 12
# speedup vs baseline: 1.4427x; 1.4427x over previous
"""Multi-head attention (B=2, S=2048, E=1024, H=16) on 8 Trainium2 cores.

Sharding: core c -> (batch b = c//4, head-group g = c%4 of 4 heads).
Each core computes Q/K/V projections for its 4 heads (256 features),
full attention for those heads, and a partial output projection
(256 rows of Wo). Host sums the 4 partials per batch element and adds bo.

On-chip layouts (per core):
  qt/kt: (128 feat-part, pair, 2048 tok)  "transposed" proj outputs; the
         128 partitions hold two heads (64+64) per pair index.
  v:     (128 tok-part, 16 tok-tiles, 4*65) per head 64 dims + a ones
         column; the ones column makes P@V_aug also produce the softmax
         denominator row.
  scores are computed transposed (key-pos on partitions, query on free)
  so exp runs on ACT along free dim and P tiles feed P@V directly as the
  moving operand; no transposes anywhere in the pipeline.
"""

import numpy as np

B, S, E, H = 2, 2048, 1024, 16
D = 64
NCORES = 8
FPC = 256  # features (head dims) per core = 4 heads

# 'f32' (exact, 4 cyc/row), 'f32r' (fp32 storage, fast PE mode), 'bf16'
MODE = "f32r"

_PROGRAMS = {}
LAST_RESULT = None
TRACE = False
TRACE_DIR = None


def _build(mode):
    import concourse.bass as bass
    import concourse.tile as tile
    from concourse import bacc, mybir

    f32 = mybir.dt.float32
    if mode == "bf16":
        DT = mybir.dt.bfloat16
    elif mode == "f32r":
        # rounded fp32 matmul format: full-rate on the PE array; every
        # producer of a matmul operand must write the f32r-typed tile
        DT = mybir.dt.float32r
    else:
        DT = f32

    def c(ap):
        return ap

    nc = bacc.Bacc("TRN2", target_bir_lowering=False, debug=False,
                   num_devices=NCORES)

    xq_ap = nc.dram_tensor("xq", [E, S], DT, kind="ExternalInput").ap()
    xk_ap = nc.dram_tensor("xk", [E, S], DT, kind="ExternalInput").ap()
    xv_ap = nc.dram_tensor("xv", [E, S], DT, kind="ExternalInput").ap()
    wq_ap = nc.dram_tensor("wq", [E, FPC], DT, kind="ExternalInput").ap()
    wk_ap = nc.dram_tensor("wk", [E, FPC], DT, kind="ExternalInput").ap()
    wv_ap = nc.dram_tensor("wv", [E, FPC], DT, kind="ExternalInput").ap()
    wo_ap = nc.dram_tensor("wo", [FPC, E], DT, kind="ExternalInput").ap()
    bqk_ap = nc.dram_tensor("bqk", [4, 128], f32, kind="ExternalInput").ap()
    bv_ap = nc.dram_tensor("bv", [1, FPC], DT, kind="ExternalInput").ap()
    ones_ap = nc.dram_tensor("ones", [1, 128], DT, kind="ExternalInput").ap()
    y_ap = nc.dram_tensor("y", [S, E], f32, kind="ExternalOutput").ap()

    Identity = mybir.ActivationFunctionType.Identity
    Exp = mybir.ActivationFunctionType.Exp

    with tile.TileContext(nc) as tc:
        with tc.tile_pool(name="persist", bufs=1) as persist:
            wq_sb = persist.tile([128, 8, FPC], DT, name="wq_sb")
            wk_sb = persist.tile([128, 8, FPC], DT, name="wk_sb")
            wv_sb = persist.tile([128, 8, FPC], DT, name="wv_sb")
            wo_sb = persist.tile([128, 2, E], DT, name="wo_sb")
            nc.sync.dma_start(wq_sb, wq_ap.rearrange("(k p) m -> p k m", p=128))
            nc.sync.dma_start(wk_sb, wk_ap.rearrange("(k p) m -> p k m", p=128))
            nc.sync.dma_start(wv_sb, wv_ap.rearrange("(k p) m -> p k m", p=128))
            nc.sync.dma_start(wo_sb, wo_ap.rearrange("(k p) m -> p k m", p=128))
            bqk_sb = persist.tile([128, 4], f32, name="bqk_sb")
            nc.sync.dma_start(bqk_sb, bqk_ap.rearrange("m p -> p m"))
            bv_sb = persist.tile([1, FPC], DT, name="bv_sb")
            nc.sync.dma_start(bv_sb, bv_ap)
            ones_sb = persist.tile([1, 128], DT, name="ones_sb")
            nc.sync.dma_start(ones_sb, ones_ap)
            ones32 = ones_sb[:, 0:64]

            qt_sb = persist.tile([128, 2, S], DT, name="qt_sb")
            kt_sb = persist.tile([128, 2, S], DT, name="kt_sb")
            v_sb = persist.tile([128, 16, 4 * 65], DT, name="v_sb")
            at_sb = persist.tile([128, 2, S], DT, name="at_sb")
            # ones column per head (index 64 of each 65-wide block)
            vre = v_sb.rearrange("p t (h e) -> p t h e", e=65)
            ones_bcast = bass.AP(
                tensor=ones_ap.tensor, offset=ones_ap.offset,
                ap=[[0, 128], [0, 64], [1, 1]])
            nc.sync.dma_start(vre[:, :, :, 64:65], ones_bcast)

            with tc.tile_pool(name="xs", bufs=3) as xpool, \
                 tc.tile_pool(name="pjps", bufs=8, space="PSUM") as pjps:
                # ---- Q and K projections: out = (feat-part, tok) ----
                for which, xap, w_sb, out_sb, bcol in (
                    (0, xq_ap, wq_sb, qt_sb, 0),
                    (1, xk_ap, wk_sb, kt_sb, 2),
                ):
                    ps = {}
                    for mt in range(2):
                        for nb in range(4):
                            ps[(mt, nb)] = pjps.tile(
                                [128, 512], f32, tag="pj",
                                name=f"pjq_{which}_{mt}_{nb}")
                    for kt in range(8):
                        xt = xpool.tile([128, S], DT, tag="x",
                                        name=f"x_{which}_{kt}")
                        nc.sync.dma_start(xt, xap[kt * 128:(kt + 1) * 128, :])
                        for mt in range(2):
                            for nb in range(4):
                                nc.tensor.matmul(
                                    ps[(mt, nb)],
                                    c(w_sb[:, kt, mt * 128:(mt + 1) * 128]),
                                    c(xt[:, nb * 512:(nb + 1) * 512]),
                                    start=(kt == 0), stop=(kt == 7))
                    for mt in range(2):
                        for nb in range(4):
                            nc.scalar.activation(
                                out_sb[:, mt, nb * 512:(nb + 1) * 512],
                                ps[(mt, nb)], Identity,
                                bias=bqk_sb[:, bcol + mt:bcol + mt + 1])

                # ---- V projection: out = (tok-part, feat), bias via K=1 mm ----
                for half in range(2):
                    psv = [pjps.tile([128, FPC], f32, tag="pj",
                                     name=f"pjv_{half}_{i}") for i in range(8)]
                    for i in range(8):
                        nc.tensor.matmul(psv[i], c(ones_sb), c(bv_sb),
                                         start=True, stop=False)
                    for kt in range(8):
                        xt = xpool.tile([128, 1024], DT, tag="x",
                                        name=f"xv_{half}_{kt}")
                        nc.sync.dma_start(
                            xt, xv_ap[kt * 128:(kt + 1) * 128,
                                      half * 1024:(half + 1) * 1024])
                        for i in range(8):
                            nc.tensor.matmul(
                                psv[i],
                                c(xt[:, i * 128:(i + 1) * 128]),
                                c(wv_sb[:, kt, :]),
                                start=False, stop=(kt == 7))
                    for i in range(8):
                        tt = half * 8 + i
                        nc.scalar.copy(
                            vre[:, tt, :, 0:64],
                            psv[i].rearrange("p (h d) -> p h d", d=64))

            # ---- attention + output projection ----
            with tc.tile_pool(name="pt", bufs=6) as ptpool, \
                 tc.tile_pool(name="sm", bufs=2) as smpool, \
                 tc.tile_pool(name="ysb", bufs=3) as ypool, \
                 tc.tile_pool(name="scps", bufs=3, space="PSUM") as scps, \
                 tc.tile_pool(name="pvps", bufs=2, space="PSUM") as pvps, \
                 tc.tile_pool(name="rbps", bufs=1, space="PSUM") as rbps, \
                 tc.tile_pool(name="yps", bufs=2, space="PSUM") as yps:
                for qb in range(4):
                    qsl = slice(qb * 512, (qb + 1) * 512)
                    for p in range(2):
                        pvt = [pvps.tile([65, 512], f32, tag="pv",
                                         name=f"pv_{qb}_{p}_{hh}")
                               for hh in range(2)]
                        for kt in range(16):
                            sc = []
                            for hh in range(2):
                                s_ = scps.tile([128, 512], f32, tag="sc",
                                               name=f"sc_{qb}_{p}_{kt}_{hh}")
                                nc.tensor.matmul(
                                    s_,
                                    c(kt_sb[64 * hh:64 * hh + 64, p,
                                            kt * 128:(kt + 1) * 128]),
                                    c(qt_sb[64 * hh:64 * hh + 64, p, qsl]),
                                    start=True, stop=True)
                                sc.append(s_)
                            for hh in range(2):
                                h = 2 * p + hh
                                ptt = ptpool.tile([128, 512], DT,
                                                  tag=f"pt{hh}",
                                                  name=f"pt_{qb}_{p}_{kt}_{hh}")
                                nc.scalar.activation(ptt, sc[hh], Exp,
                                                     scale=0.125)
                                nc.tensor.matmul(
                                    pvt[hh],
                                    c(v_sb[:, kt, 65 * h:65 * h + 65]),
                                    c(ptt),
                                    start=(kt == 0), stop=(kt == 15))
                        for hh in range(2):
                            recip = smpool.tile([1, 512], DT, tag="recip",
                                                name=f"rc_{qb}_{p}_{hh}")
                            with nc.allow_low_precision(
                                    reason="recip of softmax denom; "
                                           "f32r rounding is ample here"):
                                nc.vector.reciprocal(recip, pvt[hh][64:65, :])
                            rb = rbps.tile([64, 512], f32, tag="rb",
                                           name=f"rb_{qb}_{p}_{hh}")
                            nc.tensor.matmul(rb, c(ones32), c(recip),
                                             start=True, stop=True)
                            rbs = smpool.tile([64, 512], f32, tag="rbs",
                                              name=f"rbs_{qb}_{p}_{hh}")
                            nc.scalar.copy(rbs, rb)
                            nc.vector.tensor_mul(
                                at_sb[64 * hh:64 * hh + 64, p, qsl],
                                pvt[hh][0:64, :], rbs)
                    # output projection for this query block's 4 token tiles
                    for mt in range(4 * qb, 4 * qb + 4):
                        yp = [yps.tile([128, 512], f32, tag="y",
                                       name=f"yp_{mt}_{nb}") for nb in range(2)]
                        for nb in range(2):
                            for p2 in range(2):
                                nc.tensor.matmul(
                                    yp[nb],
                                    c(at_sb[:, p2, mt * 128:(mt + 1) * 128]),
                                    c(wo_sb[:, p2, nb * 512:(nb + 1) * 512]),
                                    start=(p2 == 0), stop=(p2 == 1))
                        yo = ypool.tile([128, E], f32, tag="yo",
                                        name=f"yo_{mt}")
                        for nb in range(2):
                            nc.vector.tensor_copy(yo[:, nb * 512:(nb + 1) * 512],
                                                  yp[nb])
                        nc.sync.dma_start(y_ap[mt * 128:(mt + 1) * 128, :], yo)

    nc.compile()
    return nc


def _get_program(mode):
    if mode not in _PROGRAMS:
        _PROGRAMS[mode] = _build(mode)
    return _PROGRAMS[mode]


def kernel(q, k, v, mask, Wq, bq, Wk, bk, Wv, bv, Wo, bo):
    global LAST_RESULT
    from concourse.bass_utils import run_bass_kernel_spmd

    mode = MODE
    nc = _get_program(mode)

    if mode == "bf16":
        import ml_dtypes
        cdt = ml_dtypes.bfloat16
    else:
        cdt = np.float32

    def prep(a, dt=None):
        return np.ascontiguousarray(a.astype(dt or cdt))

    q = np.asarray(q); k = np.asarray(k); v = np.asarray(v)
    Wq = np.asarray(Wq); Wk = np.asarray(Wk); Wv = np.asarray(Wv)
    Wo = np.asarray(Wo)
    bq = np.asarray(bq); bk = np.asarray(bk); bv = np.asarray(bv)
    bo = np.asarray(bo)

    xT = [[prep(q[b].T), prep(k[b].T), prep(v[b].T)] for b in range(B)]

    in_maps = []
    for core in range(NCORES):
        b, g = core // 4, core % 4
        r0 = g * FPC
        in_maps.append({
            "xq": xT[b][0], "xk": xT[b][1], "xv": xT[b][2],
            "wq": prep(Wq[r0:r0 + FPC, :].T),
            "wk": prep(Wk[r0:r0 + FPC, :].T),
            "wv": prep(Wv[r0:r0 + FPC, :].T),
            "wo": prep(Wo[:, r0:r0 + FPC].T),
            "bqk": np.stack([bq[r0:r0 + 128], bq[r0 + 128:r0 + FPC],
                             bk[r0:r0 + 128], bk[r0 + 128:r0 + FPC]]
                            ).astype(np.float32),
            "bv": prep(bv[r0:r0 + FPC][None, :]),
            "ones": np.ones((1, 128), cdt),
        })

    kwargs = {}
    if TRACE:
        kwargs = {"trace": True, "tmpdir": TRACE_DIR}
    res = run_bass_kernel_spmd(nc, in_maps, list(range(NCORES)), **kwargs)
    LAST_RESULT = res

    y = np.zeros((B, S, E), np.float32)
    for core in range(NCORES):
        y[core // 4] += res.results[core]["y"]
    y += bo.astype(np.float32)
    return y


# revision 16
# speedup vs baseline: 1.4960x; 1.0369x over previous
"""Multi-head attention (B=2, S=2048, E=1024, H=16) on 8 Trainium2 cores.

Sharding: core c -> (batch b = c//4, head-group g = c%4 of 4 heads).
Each core computes Q/K/V projections for its 4 heads (256 features),
full attention for those heads, and a partial output projection
(256 rows of Wo). Host sums the 4 partials per batch element and adds bo.

On-chip layouts (per core):
  qt/kt: (128 feat-part, pair, 2048 tok)  "transposed" proj outputs; the
         128 partitions hold two heads (64+64) per pair index.
  v:     (128 tok-part, 16 tok-tiles, 4*65) per head 64 dims + a ones
         column; the ones column makes P@V_aug also produce the softmax
         denominator row.
  scores are computed transposed (key-pos on partitions, query on free)
  so exp runs on ACT along free dim and P tiles feed P@V directly as the
  moving operand; no transposes anywhere in the pipeline.
"""

import numpy as np

B, S, E, H = 2, 2048, 1024, 16
D = 64
NCORES = 8
FPC = 256  # features (head dims) per core = 4 heads

# 'f32' (exact, 4 cyc/row), 'f32r' (fp32 storage, fast PE mode), 'bf16'
MODE = "f32r"

_PROGRAMS = {}
LAST_RESULT = None
TRACE = False
TRACE_DIR = None


def _build(mode):
    import concourse.bass as bass
    import concourse.tile as tile
    from concourse import bacc, mybir

    f32 = mybir.dt.float32
    if mode == "bf16":
        DT = mybir.dt.bfloat16
    elif mode == "f32r":
        # rounded fp32 matmul format: full-rate on the PE array; every
        # producer of a matmul operand must write the f32r-typed tile
        DT = mybir.dt.float32r
    else:
        DT = f32

    def c(ap):
        return ap

    nc = bacc.Bacc("TRN2", target_bir_lowering=False, debug=False,
                   num_devices=NCORES)

    xq_ap = nc.dram_tensor("xq", [E, S], DT, kind="ExternalInput").ap()
    xk_ap = nc.dram_tensor("xk", [E, S], DT, kind="ExternalInput").ap()
    xv_ap = nc.dram_tensor("xv", [E, S], DT, kind="ExternalInput").ap()
    wq_ap = nc.dram_tensor("wq", [E, FPC], DT, kind="ExternalInput").ap()
    wk_ap = nc.dram_tensor("wk", [E, FPC], DT, kind="ExternalInput").ap()
    wv_ap = nc.dram_tensor("wv", [E, FPC], DT, kind="ExternalInput").ap()
    wo_ap = nc.dram_tensor("wo", [FPC, E], DT, kind="ExternalInput").ap()
    bqk_ap = nc.dram_tensor("bqk", [4, 128], f32, kind="ExternalInput").ap()
    bv_ap = nc.dram_tensor("bv", [1, FPC], DT, kind="ExternalInput").ap()
    ones_ap = nc.dram_tensor("ones", [1, 128], DT, kind="ExternalInput").ap()
    y_ap = nc.dram_tensor("y", [S, E], f32, kind="ExternalOutput").ap()

    Identity = mybir.ActivationFunctionType.Identity
    Exp = mybir.ActivationFunctionType.Exp

    with tile.TileContext(nc) as tc:
        with tc.tile_pool(name="persist", bufs=1) as persist:
            wq_sb = persist.tile([128, 8, FPC], DT, name="wq_sb")
            wk_sb = persist.tile([128, 8, FPC], DT, name="wk_sb")
            wv_sb = persist.tile([128, 8, FPC], DT, name="wv_sb")
            wo_sb = persist.tile([128, 2, E], DT, name="wo_sb")
            nc.sync.dma_start(wq_sb, wq_ap.rearrange("(k p) m -> p k m", p=128))
            nc.sync.dma_start(wk_sb, wk_ap.rearrange("(k p) m -> p k m", p=128))
            nc.sync.dma_start(wv_sb, wv_ap.rearrange("(k p) m -> p k m", p=128))
            nc.sync.dma_start(wo_sb, wo_ap.rearrange("(k p) m -> p k m", p=128))
            bqk_sb = persist.tile([128, 4], f32, name="bqk_sb")
            nc.sync.dma_start(bqk_sb, bqk_ap.rearrange("m p -> p m"))
            bv_sb = persist.tile([1, FPC], DT, name="bv_sb")
            nc.sync.dma_start(bv_sb, bv_ap)
            ones_sb = persist.tile([1, 128], DT, name="ones_sb")
            nc.sync.dma_start(ones_sb, ones_ap)
            ones32 = ones_sb[:, 0:64]

            qt_sb = persist.tile([128, 2, S], DT, name="qt_sb")
            kt_sb = persist.tile([128, 2, S], DT, name="kt_sb")
            v_sb = persist.tile([128, 16, 4 * 65], DT, name="v_sb")
            at_sb = persist.tile([128, 2, S], DT, name="at_sb")
            # ones column per head (index 64 of each 65-wide block)
            vre = v_sb.rearrange("p t (h e) -> p t h e", e=65)
            ones_bcast = bass.AP(
                tensor=ones_ap.tensor, offset=ones_ap.offset,
                ap=[[0, 128], [0, 64], [1, 1]])
            nc.sync.dma_start(vre[:, :, :, 64:65], ones_bcast)

            with tc.tile_pool(name="xs", bufs=3) as xpool, \
                 tc.tile_pool(name="pjps", bufs=8, space="PSUM") as pjps:
                # ---- Q and K projections: out = (feat-part, tok) ----
                for which, xap, w_sb, out_sb, bcol in (
                    (0, xq_ap, wq_sb, qt_sb, 0),
                    (1, xk_ap, wk_sb, kt_sb, 2),
                ):
                    ps = {}
                    for mt in range(2):
                        for nb in range(4):
                            ps[(mt, nb)] = pjps.tile(
                                [128, 512], f32, tag="pj",
                                name=f"pjq_{which}_{mt}_{nb}")
                    for kt in range(8):
                        xt = xpool.tile([128, S], DT, tag="x",
                                        name=f"x_{which}_{kt}")
                        nc.sync.dma_start(xt, xap[kt * 128:(kt + 1) * 128, :])
                        for mt in range(2):
                            for nb in range(4):
                                nc.tensor.matmul(
                                    ps[(mt, nb)],
                                    c(w_sb[:, kt, mt * 128:(mt + 1) * 128]),
                                    c(xt[:, nb * 512:(nb + 1) * 512]),
                                    start=(kt == 0), stop=(kt == 7))
                    for mt in range(2):
                        for nb in range(4):
                            nc.vector.tensor_scalar_add(
                                out_sb[:, mt, nb * 512:(nb + 1) * 512],
                                ps[(mt, nb)],
                                bqk_sb[:, bcol + mt:bcol + mt + 1])

                # ---- V projection: out = (tok-part, feat), bias via K=1 mm ----
                for half in range(2):
                    psv = [pjps.tile([128, FPC], f32, tag="pj",
                                     name=f"pjv_{half}_{i}") for i in range(8)]
                    for i in range(8):
                        nc.tensor.matmul(psv[i], c(ones_sb), c(bv_sb),
                                         start=True, stop=False)
                    for kt in range(8):
                        xt = xpool.tile([128, 1024], DT, tag="x",
                                        name=f"xv_{half}_{kt}")
                        nc.sync.dma_start(
                            xt, xv_ap[kt * 128:(kt + 1) * 128,
                                      half * 1024:(half + 1) * 1024])
                        for i in range(8):
                            nc.tensor.matmul(
                                psv[i],
                                c(xt[:, i * 128:(i + 1) * 128]),
                                c(wv_sb[:, kt, :]),
                                start=False, stop=(kt == 7))
                    for i in range(8):
                        tt = half * 8 + i
                        nc.vector.tensor_copy(
                            vre[:, tt, :, 0:64],
                            psv[i].rearrange("p (h d) -> p h d", d=64))

            # ---- attention + output projection ----
            with tc.tile_pool(name="pt", bufs=8) as ptpool, \
                 tc.tile_pool(name="sm", bufs=2) as smpool, \
                 tc.tile_pool(name="ysb", bufs=3) as ypool, \
                 tc.tile_pool(name="scps", bufs=3, space="PSUM") as scps, \
                 tc.tile_pool(name="pvps", bufs=2, space="PSUM") as pvps, \
                 tc.tile_pool(name="rbps", bufs=1, space="PSUM") as rbps, \
                 tc.tile_pool(name="yps", bufs=2, space="PSUM") as yps:
                for qb in range(4):
                    qsl = slice(qb * 512, (qb + 1) * 512)
                    for p in range(2):
                        pvt = [pvps.tile([65, 512], f32, tag="pv",
                                         name=f"pv_{qb}_{p}_{hh}")
                               for hh in range(2)]
                        for kt in range(16):
                            sc = []
                            for hh in range(2):
                                s_ = scps.tile([128, 512], f32, tag="sc",
                                               name=f"sc_{qb}_{p}_{kt}_{hh}")
                                nc.tensor.matmul(
                                    s_,
                                    c(kt_sb[64 * hh:64 * hh + 64, p,
                                            kt * 128:(kt + 1) * 128]),
                                    c(qt_sb[64 * hh:64 * hh + 64, p, qsl]),
                                    start=True, stop=True)
                                sc.append(s_)
                            for hh in range(2):
                                h = 2 * p + hh
                                ptt = ptpool.tile([128, 512], DT,
                                                  tag=f"pt{hh}",
                                                  name=f"pt_{qb}_{p}_{kt}_{hh}")
                                nc.scalar.activation(ptt, sc[hh], Exp,
                                                     scale=0.125)
                                nc.tensor.matmul(
                                    pvt[hh],
                                    c(v_sb[:, kt, 65 * h:65 * h + 65]),
                                    c(ptt),
                                    start=(kt == 0), stop=(kt == 15))
                        for hh in range(2):
                            # denom row -> SBUF, broadcast to 64 partitions
                            # via ones-matmul, then full-width reciprocal
                            denr = smpool.tile([1, 512], DT, tag="denr",
                                               name=f"dn_{qb}_{p}_{hh}")
                            nc.vector.tensor_copy(denr, pvt[hh][64:65, :])
                            rb = rbps.tile([64, 512], f32, tag="rb",
                                           name=f"rb_{qb}_{p}_{hh}")
                            nc.tensor.matmul(rb, c(ones32), c(denr),
                                             start=True, stop=True)
                            rbs = smpool.tile([64, 512], f32, tag="rbs",
                                              name=f"rbs_{qb}_{p}_{hh}")
                            nc.vector.reciprocal(rbs, rb)
                            nc.vector.tensor_mul(
                                at_sb[64 * hh:64 * hh + 64, p, qsl],
                                pvt[hh][0:64, :], rbs)
                    # output projection for this query block's 4 token tiles
                    for mt in range(4 * qb, 4 * qb + 4):
                        yp = [yps.tile([128, 512], f32, tag="y",
                                       name=f"yp_{mt}_{nb}") for nb in range(2)]
                        for nb in range(2):
                            for p2 in range(2):
                                nc.tensor.matmul(
                                    yp[nb],
                                    c(at_sb[:, p2, mt * 128:(mt + 1) * 128]),
                                    c(wo_sb[:, p2, nb * 512:(nb + 1) * 512]),
                                    start=(p2 == 0), stop=(p2 == 1))
                        yo = ypool.tile([128, E], f32, tag="yo",
                                        name=f"yo_{mt}")
                        for nb in range(2):
                            nc.vector.tensor_copy(yo[:, nb * 512:(nb + 1) * 512],
                                                  yp[nb])
                        nc.sync.dma_start(y_ap[mt * 128:(mt + 1) * 128, :], yo)

    nc.compile()
    return nc


def _get_program(mode):
    if mode not in _PROGRAMS:
        _PROGRAMS[mode] = _build(mode)
    return _PROGRAMS[mode]


def kernel(q, k, v, mask, Wq, bq, Wk, bk, Wv, bv, Wo, bo):
    global LAST_RESULT
    from concourse.bass_utils import run_bass_kernel_spmd

    mode = MODE
    nc = _get_program(mode)

    if mode == "bf16":
        import ml_dtypes
        cdt = ml_dtypes.bfloat16
    else:
        cdt = np.float32

    def prep(a, dt=None):
        return np.ascontiguousarray(a.astype(dt or cdt))

    q = np.asarray(q); k = np.asarray(k); v = np.asarray(v)
    Wq = np.asarray(Wq); Wk = np.asarray(Wk); Wv = np.asarray(Wv)
    Wo = np.asarray(Wo)
    bq = np.asarray(bq); bk = np.asarray(bk); bv = np.asarray(bv)
    bo = np.asarray(bo)

    xT = [[prep(q[b].T), prep(k[b].T), prep(v[b].T)] for b in range(B)]

    in_maps = []
    for core in range(NCORES):
        b, g = core // 4, core % 4
        r0 = g * FPC
        in_maps.append({
            "xq": xT[b][0], "xk": xT[b][1], "xv": xT[b][2],
            "wq": prep(Wq[r0:r0 + FPC, :].T),
            "wk": prep(Wk[r0:r0 + FPC, :].T),
            "wv": prep(Wv[r0:r0 + FPC, :].T),
            "wo": prep(Wo[:, r0:r0 + FPC].T),
            "bqk": np.stack([bq[r0:r0 + 128], bq[r0 + 128:r0 + FPC],
                             bk[r0:r0 + 128], bk[r0 + 128:r0 + FPC]]
                            ).astype(np.float32),
            "bv": prep(bv[r0:r0 + FPC][None, :]),
            "ones": np.ones((1, 128), cdt),
        })

    kwargs = {}
    if TRACE:
        kwargs = {"trace": True, "tmpdir": TRACE_DIR}
    res = run_bass_kernel_spmd(nc, in_maps, list(range(NCORES)), **kwargs)
    LAST_RESULT = res

    y = np.zeros((B, S, E), np.float32)
    for core in range(NCORES):
        y[core // 4] += res.results[core]["y"]
    y += bo.astype(np.float32)
    return y


# revision 17
# speedup vs baseline: 2.0488x; 1.3695x over previous
"""Multi-head attention (B=2, S=2048, E=1024, H=16) on 8 Trainium2 cores.

Sharding: core c -> (batch b = c//4, head-group g = c%4 of 4 heads).
Each core computes Q/K/V projections for its 4 heads (256 features),
full attention for those heads, and a partial output projection
(256 rows of Wo). Host sums the 4 partials per batch element and adds bo.

On-chip layouts (per core):
  qt/kt: (128 feat-part, pair, 2048 tok)  transposed proj outputs; the
         128 partitions hold two heads (64+64) per pair index.
  v:     (128 tok-part, 16 tok-tiles, 4*65): per head 64 dims plus a
         "ones" column produced by an augmented V projection (extra
         output feature with zero weights and bias 1.0); P @ V_aug then
         also yields the softmax denominator row for free.
  scores are computed transposed (key-pos on partitions, query on free)
  so exp runs on ACT along the free dim and P tiles feed P@V directly as
  the moving operand; no transposes anywhere in the pipeline.

All weight matrices are re-laid-out on the host so every DMA is a
contiguous per-partition run (the partition-strided gather variant threw
thousands of 1KB packets at one DGE queue and starved the PE for 100us).
"""

import numpy as np

B, S, E, H = 2, 2048, 1024, 16
D = 64
NCORES = 8
FPC = 256  # features (head dims) per core = 4 heads
VW = 4 * 65  # V-projection output width incl. ones columns

# 'f32' (exact, 4 cyc/row), 'f32r' (fp32 storage, rounded fast PE mode), 'bf16'
MODE = "f32r"

_PROGRAMS = {}
LAST_RESULT = None
TRACE = False
TRACE_DIR = None


def _build(mode):
    import concourse.tile as tile
    from concourse import bacc, mybir

    f32 = mybir.dt.float32
    if mode == "bf16":
        DT = mybir.dt.bfloat16
    elif mode == "f32r":
        DT = mybir.dt.float32r
    else:
        DT = f32

    def c(ap):
        return ap

    nc = bacc.Bacc("TRN2", target_bir_lowering=False, debug=False,
                   num_devices=NCORES)

    xq_ap = nc.dram_tensor("xq", [E, S], DT, kind="ExternalInput").ap()
    xk_ap = nc.dram_tensor("xk", [E, S], DT, kind="ExternalInput").ap()
    xv_ap = nc.dram_tensor("xv", [E, S], DT, kind="ExternalInput").ap()
    wq_ap = nc.dram_tensor("wq", [128, 8, FPC], DT, kind="ExternalInput").ap()
    wk_ap = nc.dram_tensor("wk", [128, 8, FPC], DT, kind="ExternalInput").ap()
    wv_ap = nc.dram_tensor("wv", [128, 8, VW], DT, kind="ExternalInput").ap()
    wo_ap = nc.dram_tensor("wo", [128, 2, E], DT, kind="ExternalInput").ap()
    bqk_ap = nc.dram_tensor("bqk", [128, 4], f32, kind="ExternalInput").ap()
    bv_ap = nc.dram_tensor("bv", [1, VW], DT, kind="ExternalInput").ap()
    ones_ap = nc.dram_tensor("ones", [1, 128], DT, kind="ExternalInput").ap()
    y_ap = nc.dram_tensor("y", [S, E], f32, kind="ExternalOutput").ap()

    Exp = mybir.ActivationFunctionType.Exp

    with tile.TileContext(nc) as tc:
        with tc.tile_pool(name="persist", bufs=1) as persist:
            wq_sb = persist.tile([128, 8, FPC], DT, name="wq_sb")
            wk_sb = persist.tile([128, 8, FPC], DT, name="wk_sb")
            wv_sb = persist.tile([128, 8, VW], DT, name="wv_sb")
            wo_sb = persist.tile([128, 2, E], DT, name="wo_sb")
            bqk_sb = persist.tile([128, 4], f32, name="bqk_sb")
            bv_sb = persist.tile([1, VW], DT, name="bv_sb")
            ones_sb = persist.tile([1, 128], DT, name="ones_sb")
            # weights/constants on the GpSimd DGE queue, x loads on Sync's:
            # the two streams run in parallel so the first projection matmul
            # isn't gated on the whole weight preload.
            nc.gpsimd.dma_start(wq_sb, wq_ap)
            nc.gpsimd.dma_start(wk_sb, wk_ap)
            nc.gpsimd.dma_start(wv_sb, wv_ap)
            nc.gpsimd.dma_start(wo_sb, wo_ap)
            nc.gpsimd.dma_start(bqk_sb, bqk_ap)
            nc.gpsimd.dma_start(bv_sb, bv_ap)
            nc.gpsimd.dma_start(ones_sb, ones_ap)
            ones32 = ones_sb[:, 0:64]

            qt_sb = persist.tile([128, 2, S], DT, name="qt_sb")
            kt_sb = persist.tile([128, 2, S], DT, name="kt_sb")
            v_sb = persist.tile([128, 16, VW], DT, name="v_sb")
            at_sb = persist.tile([128, 2, S], DT, name="at_sb")

            with tc.tile_pool(name="xs", bufs=3) as xpool, \
                 tc.tile_pool(name="pjps", bufs=8, space="PSUM") as pjps:
                # ---- Q and K projections: out = (feat-part, tok) ----
                for which, xap, w_sb, out_sb, bcol in (
                    (0, xq_ap, wq_sb, qt_sb, 0),
                    (1, xk_ap, wk_sb, kt_sb, 2),
                ):
                    ps = {}
                    for mt in range(2):
                        for nb in range(4):
                            ps[(mt, nb)] = pjps.tile(
                                [128, 512], f32, tag="pj",
                                name=f"pjq_{which}_{mt}_{nb}")
                    for kt in range(8):
                        xt = xpool.tile([128, S], DT, tag="x",
                                        name=f"x_{which}_{kt}")
                        nc.sync.dma_start(xt, xap[kt * 128:(kt + 1) * 128, :])
                        for mt in range(2):
                            for nb in range(4):
                                nc.tensor.matmul(
                                    ps[(mt, nb)],
                                    c(w_sb[:, kt, mt * 128:(mt + 1) * 128]),
                                    c(xt[:, nb * 512:(nb + 1) * 512]),
                                    start=(kt == 0), stop=(kt == 7))
                    for mt in range(2):
                        for nb in range(4):
                            nc.vector.tensor_scalar_add(
                                out_sb[:, mt, nb * 512:(nb + 1) * 512],
                                ps[(mt, nb)],
                                bqk_sb[:, bcol + mt:bcol + mt + 1])

                # ---- V projection: out = (tok-part, 4*65 feat) ----
                # bias row via K=1 ones-matmul; the augmented columns carry
                # zero weights + bias 1.0 -> ones columns for the denominator
                for half in range(2):
                    psv = [pjps.tile([128, VW], f32, tag="pj",
                                     name=f"pjv_{half}_{i}") for i in range(8)]
                    for i in range(8):
                        nc.tensor.matmul(psv[i], c(ones_sb), c(bv_sb),
                                         start=True, stop=False)
                    for kt in range(8):
                        xt = xpool.tile([128, 1024], DT, tag="x",
                                        name=f"xv_{half}_{kt}")
                        nc.sync.dma_start(
                            xt, xv_ap[kt * 128:(kt + 1) * 128,
                                      half * 1024:(half + 1) * 1024])
                        for i in range(8):
                            nc.tensor.matmul(
                                psv[i],
                                c(xt[:, i * 128:(i + 1) * 128]),
                                c(wv_sb[:, kt, :]),
                                start=False, stop=(kt == 7))
                    for i in range(8):
                        tt = half * 8 + i
                        nc.vector.tensor_copy(v_sb[:, tt, :], psv[i])

            # ---- attention + output projection ----
            with tc.tile_pool(name="pt", bufs=8) as ptpool, \
                 tc.tile_pool(name="sm", bufs=2) as smpool, \
                 tc.tile_pool(name="ysb", bufs=3) as ypool, \
                 tc.tile_pool(name="scps", bufs=3, space="PSUM") as scps, \
                 tc.tile_pool(name="pvps", bufs=2, space="PSUM") as pvps, \
                 tc.tile_pool(name="rbps", bufs=1, space="PSUM") as rbps, \
                 tc.tile_pool(name="yps", bufs=2, space="PSUM") as yps:
                for qb in range(4):
                    qsl = slice(qb * 512, (qb + 1) * 512)
                    for p in range(2):
                        pvt = [pvps.tile([65, 512], f32, tag="pv",
                                         name=f"pv_{qb}_{p}_{hh}")
                               for hh in range(2)]
                        for kt in range(16):
                            sc = []
                            for hh in range(2):
                                s_ = scps.tile([128, 512], f32, tag="sc",
                                               name=f"sc_{qb}_{p}_{kt}_{hh}")
                                nc.tensor.matmul(
                                    s_,
                                    c(kt_sb[64 * hh:64 * hh + 64, p,
                                            kt * 128:(kt + 1) * 128]),
                                    c(qt_sb[64 * hh:64 * hh + 64, p, qsl]),
                                    start=True, stop=True)
                                sc.append(s_)
                            for hh in range(2):
                                h = 2 * p + hh
                                ptt = ptpool.tile([128, 512], DT,
                                                  tag=f"pt{hh}",
                                                  name=f"pt_{qb}_{p}_{kt}_{hh}")
                                nc.scalar.activation(ptt, sc[hh], Exp,
                                                     scale=0.125)
                                nc.tensor.matmul(
                                    pvt[hh],
                                    c(v_sb[:, kt, 65 * h:65 * h + 65]),
                                    c(ptt),
                                    start=(kt == 0), stop=(kt == 15))
                        for hh in range(2):
                            # denom row -> SBUF, broadcast to 64 partitions
                            # via ones-matmul, then full-width reciprocal
                            denr = smpool.tile([1, 512], DT, tag="denr",
                                               name=f"dn_{qb}_{p}_{hh}")
                            nc.vector.tensor_copy(denr, pvt[hh][64:65, :])
                            rb = rbps.tile([64, 512], f32, tag="rb",
                                           name=f"rb_{qb}_{p}_{hh}")
                            nc.tensor.matmul(rb, c(ones32), c(denr),
                                             start=True, stop=True)
                            rbs = smpool.tile([64, 512], f32, tag="rbs",
                                              name=f"rbs_{qb}_{p}_{hh}")
                            nc.vector.reciprocal(rbs, rb)
                            nc.vector.tensor_mul(
                                at_sb[64 * hh:64 * hh + 64, p, qsl],
                                pvt[hh][0:64, :], rbs)
                    # output projection for this query block's 4 token tiles
                    for mt in range(4 * qb, 4 * qb + 4):
                        yp = [yps.tile([128, 512], f32, tag="y",
                                       name=f"yp_{mt}_{nb}") for nb in range(2)]
                        for nb in range(2):
                            for p2 in range(2):
                                nc.tensor.matmul(
                                    yp[nb],
                                    c(at_sb[:, p2, mt * 128:(mt + 1) * 128]),
                                    c(wo_sb[:, p2, nb * 512:(nb + 1) * 512]),
                                    start=(p2 == 0), stop=(p2 == 1))
                        yo = ypool.tile([128, E], f32, tag="yo",
                                        name=f"yo_{mt}")
                        for nb in range(2):
                            nc.vector.tensor_copy(yo[:, nb * 512:(nb + 1) * 512],
                                                  yp[nb])
                        nc.gpsimd.dma_start(y_ap[mt * 128:(mt + 1) * 128, :], yo)

    nc.compile()
    return nc


def _get_program(mode):
    if mode not in _PROGRAMS:
        _PROGRAMS[mode] = _build(mode)
    return _PROGRAMS[mode]


def kernel(q, k, v, mask, Wq, bq, Wk, bk, Wv, bv, Wo, bo):
    global LAST_RESULT
    from concourse.bass_utils import run_bass_kernel_spmd

    mode = MODE
    nc = _get_program(mode)

    if mode == "bf16":
        import ml_dtypes
        cdt = ml_dtypes.bfloat16
    else:
        cdt = np.float32

    def prep(a):
        return np.ascontiguousarray(np.asarray(a).astype(cdt))

    q = np.asarray(q); k = np.asarray(k); v = np.asarray(v)
    Wq = np.asarray(Wq); Wk = np.asarray(Wk); Wv = np.asarray(Wv)
    Wo = np.asarray(Wo)
    bq = np.asarray(bq); bk = np.asarray(bk); bv = np.asarray(bv)
    bo = np.asarray(bo)

    xT = [[prep(q[b].T), prep(k[b].T), prep(v[b].T)] for b in range(B)]

    in_maps = []
    for core in range(NCORES):
        b, g = core // 4, core % 4
        r0 = g * FPC

        def wqk_layout(W):
            # lhsT tiles: [part p, ktile, m] = W.T[kt*128+p, m]
            A = W[r0:r0 + FPC, :].T.reshape(8, 128, FPC)
            return prep(A.transpose(1, 0, 2))

        WvT = Wv[r0:r0 + FPC, :].T  # (E, 256)
        Wv_aug = np.zeros((E, VW), np.float32)
        bv_aug = np.zeros((1, VW), np.float32)
        for h in range(4):
            Wv_aug[:, 65 * h:65 * h + 64] = WvT[:, 64 * h:64 * h + 64]
            bv_aug[0, 65 * h:65 * h + 64] = bv[r0 + 64 * h:r0 + 64 * h + 64]
            bv_aug[0, 65 * h + 64] = 1.0
        Wo_l = Wo[:, r0:r0 + FPC].T.reshape(2, 128, E).transpose(1, 0, 2)

        in_maps.append({
            "xq": xT[b][0], "xk": xT[b][1], "xv": xT[b][2],
            "wq": wqk_layout(Wq),
            "wk": wqk_layout(Wk),
            "wv": prep(Wv_aug.reshape(8, 128, VW).transpose(1, 0, 2)),
            "wo": prep(Wo_l),
            "bqk": np.stack([bq[r0:r0 + 128], bq[r0 + 128:r0 + FPC],
                             bk[r0:r0 + 128], bk[r0 + 128:r0 + FPC]],
                            axis=1).astype(np.float32),
            "bv": prep(bv_aug),
            "ones": np.ones((1, 128), cdt),
        })

    kwargs = {}
    if TRACE:
        kwargs = {"trace": True, "tmpdir": TRACE_DIR}
    res = run_bass_kernel_spmd(nc, in_maps, list(range(NCORES)), **kwargs)
    LAST_RESULT = res

    y = np.zeros((B, S, E), np.float32)
    for core in range(NCORES):
        y[core // 4] += res.results[core]["y"]
    y += bo.astype(np.float32)
    return y


# revision 18
# speedup vs baseline: 2.0870x; 1.0187x over previous
"""Multi-head attention (B=2, S=2048, E=1024, H=16) on 8 Trainium2 cores.

Sharding: core c -> (batch b = c//4, head-group g = c%4 of 4 heads).
Each core computes Q/K/V projections for its 4 heads (256 features),
full attention for those heads, and a partial output projection
(256 rows of Wo). Host sums the 4 partials per batch element and adds bo.

On-chip layouts (per core):
  qt/kt: (128 feat-part, pair, 2048 tok)  transposed proj outputs; the
         128 partitions hold two heads (64+64) per pair index.
  v:     (128 tok-part, 16 tok-tiles, 4*65): per head 64 dims plus a
         "ones" column produced by an augmented V projection (extra
         output feature with zero weights and bias 1.0); P @ V_aug then
         also yields the softmax denominator row for free.
  scores are computed transposed (key-pos on partitions, query on free)
  so exp runs on ACT along the free dim and P tiles feed P@V directly as
  the moving operand; no transposes anywhere in the pipeline.

All weight matrices are re-laid-out on the host so every DMA is a
contiguous per-partition run (the partition-strided gather variant threw
thousands of 1KB packets at one DGE queue and starved the PE for 100us).
"""

import numpy as np

B, S, E, H = 2, 2048, 1024, 16
D = 64
NCORES = 8
FPC = 256  # features (head dims) per core = 4 heads
VW = 4 * 65  # V-projection output width incl. ones columns

# 'f32' (exact, 4 cyc/row), 'f32r' (fp32 storage, rounded fast PE mode), 'bf16'
MODE = "f32r"

_PROGRAMS = {}
LAST_RESULT = None
TRACE = False
TRACE_DIR = None


def _build(mode):
    import concourse.tile as tile
    from concourse import bacc, mybir

    f32 = mybir.dt.float32
    if mode == "bf16":
        DT = mybir.dt.bfloat16
    elif mode == "f32r":
        DT = mybir.dt.float32r
    else:
        DT = f32

    def c(ap):
        return ap

    nc = bacc.Bacc("TRN2", target_bir_lowering=False, debug=False,
                   num_devices=NCORES)

    xq_ap = nc.dram_tensor("xq", [E, S], DT, kind="ExternalInput").ap()
    xk_ap = nc.dram_tensor("xk", [E, S], DT, kind="ExternalInput").ap()
    xv_ap = nc.dram_tensor("xv", [E, S], DT, kind="ExternalInput").ap()
    wq_ap = nc.dram_tensor("wq", [128, 8, FPC], DT, kind="ExternalInput").ap()
    wk_ap = nc.dram_tensor("wk", [128, 8, FPC], DT, kind="ExternalInput").ap()
    wv_ap = nc.dram_tensor("wv", [128, 8, VW], DT, kind="ExternalInput").ap()
    wo_ap = nc.dram_tensor("wo", [128, 2, E], DT, kind="ExternalInput").ap()
    bqk_ap = nc.dram_tensor("bqk", [128, 4], f32, kind="ExternalInput").ap()
    bv_ap = nc.dram_tensor("bv", [1, VW], DT, kind="ExternalInput").ap()
    ones_ap = nc.dram_tensor("ones", [1, 128], DT, kind="ExternalInput").ap()
    y_ap = nc.dram_tensor("y", [S, E], f32, kind="ExternalOutput").ap()

    Exp = mybir.ActivationFunctionType.Exp

    with tile.TileContext(nc) as tc:
        with tc.tile_pool(name="persist", bufs=1) as persist:
            wq_sb = persist.tile([128, 8, FPC], DT, name="wq_sb")
            wk_sb = persist.tile([128, 8, FPC], DT, name="wk_sb")
            wv_sb = persist.tile([128, 8, VW], DT, name="wv_sb")
            wo_sb = persist.tile([128, 2, E], DT, name="wo_sb")
            bqk_sb = persist.tile([128, 4], f32, name="bqk_sb")
            bv_sb = persist.tile([1, VW], DT, name="bv_sb")
            ones_sb = persist.tile([1, 128], DT, name="ones_sb")
            # weights/constants on the GpSimd DGE queue, x loads on Sync's:
            # the two streams run in parallel so the first projection matmul
            # isn't gated on the whole weight preload.
            nc.gpsimd.dma_start(wq_sb, wq_ap)
            nc.gpsimd.dma_start(wk_sb, wk_ap)
            nc.gpsimd.dma_start(wv_sb, wv_ap)
            nc.gpsimd.dma_start(wo_sb, wo_ap)
            nc.gpsimd.dma_start(bqk_sb, bqk_ap)
            nc.gpsimd.dma_start(bv_sb, bv_ap)
            nc.gpsimd.dma_start(ones_sb, ones_ap)
            ones32 = ones_sb[:, 0:64]

            qt_sb = persist.tile([128, 2, S], DT, name="qt_sb")
            kt_sb = persist.tile([128, 2, S], DT, name="kt_sb")
            v_sb = persist.tile([128, 16, VW], DT, name="v_sb")
            at_sb = persist.tile([128, 2, S], DT, name="at_sb")

            with tc.tile_pool(name="xs", bufs=3) as xpool, \
                 tc.tile_pool(name="pjps", bufs=8, space="PSUM") as pjps:
                # ---- Q and K projections: out = (feat-part, tok) ----
                for which, xap, w_sb, out_sb, bcol in (
                    (0, xq_ap, wq_sb, qt_sb, 0),
                    (1, xk_ap, wk_sb, kt_sb, 2),
                ):
                    ps = {}
                    for mt in range(2):
                        for nb in range(4):
                            ps[(mt, nb)] = pjps.tile(
                                [128, 512], f32, tag="pj",
                                name=f"pjq_{which}_{mt}_{nb}")
                    for kt in range(8):
                        xt = xpool.tile([128, S], DT, tag="x",
                                        name=f"x_{which}_{kt}")
                        nc.sync.dma_start(xt, xap[kt * 128:(kt + 1) * 128, :])
                        for mt in range(2):
                            for nb in range(4):
                                nc.tensor.matmul(
                                    ps[(mt, nb)],
                                    c(w_sb[:, kt, mt * 128:(mt + 1) * 128]),
                                    c(xt[:, nb * 512:(nb + 1) * 512]),
                                    start=(kt == 0), stop=(kt == 7))
                    for mt in range(2):
                        for nb in range(4):
                            nc.vector.tensor_scalar_add(
                                out_sb[:, mt, nb * 512:(nb + 1) * 512],
                                ps[(mt, nb)],
                                bqk_sb[:, bcol + mt:bcol + mt + 1])

                # ---- V projection: out = (tok-part, 4*65 feat) ----
                # bias row via K=1 ones-matmul; the augmented columns carry
                # zero weights + bias 1.0 -> ones columns for the denominator
                for half in range(2):
                    psv = [pjps.tile([128, VW], f32, tag="pj",
                                     name=f"pjv_{half}_{i}") for i in range(8)]
                    for i in range(8):
                        nc.tensor.matmul(psv[i], c(ones_sb), c(bv_sb),
                                         start=True, stop=False)
                    for kt in range(8):
                        xt = xpool.tile([128, 1024], DT, tag="x",
                                        name=f"xv_{half}_{kt}")
                        nc.sync.dma_start(
                            xt, xv_ap[kt * 128:(kt + 1) * 128,
                                      half * 1024:(half + 1) * 1024])
                        for i in range(8):
                            nc.tensor.matmul(
                                psv[i],
                                c(xt[:, i * 128:(i + 1) * 128]),
                                c(wv_sb[:, kt, :]),
                                start=False, stop=(kt == 7))
                    for i in range(8):
                        tt = half * 8 + i
                        nc.vector.tensor_copy(v_sb[:, tt, :], psv[i])

            # ---- attention + output projection ----
            with tc.tile_pool(name="pt", bufs=8) as ptpool, \
                 tc.tile_pool(name="sm", bufs=2) as smpool, \
                 tc.tile_pool(name="ysb", bufs=3) as ypool, \
                 tc.tile_pool(name="scps", bufs=2, space="PSUM") as scps, \
                 tc.tile_pool(name="pvps", bufs=2, space="PSUM") as pvps, \
                 tc.tile_pool(name="rbps", bufs=1, space="PSUM") as rbps, \
                 tc.tile_pool(name="yps", bufs=1, space="PSUM") as yps:
                for qb in range(4):
                    qsl = slice(qb * 512, (qb + 1) * 512)
                    for p in range(2):
                        pvt = [pvps.tile([65, 512], f32, tag="pv",
                                         name=f"pv_{qb}_{p}_{hh}")
                               for hh in range(2)]
                        for kt in range(16):
                            # both heads' transposed scores in one 2-bank
                            # tile; a single exp covers the pair
                            s_ = scps.tile([128, 1024], f32, tag="sc",
                                           name=f"sc_{qb}_{p}_{kt}")
                            for hh in range(2):
                                nc.tensor.matmul(
                                    s_[:, 512 * hh:512 * hh + 512],
                                    c(kt_sb[64 * hh:64 * hh + 64, p,
                                            kt * 128:(kt + 1) * 128]),
                                    c(qt_sb[64 * hh:64 * hh + 64, p, qsl]),
                                    start=True, stop=True)
                            ptt = ptpool.tile([128, 1024], DT, tag="pt",
                                              name=f"pt_{qb}_{p}_{kt}")
                            nc.scalar.activation(ptt, s_, Exp, scale=0.125)
                            for hh in range(2):
                                h = 2 * p + hh
                                nc.tensor.matmul(
                                    pvt[hh],
                                    c(v_sb[:, kt, 65 * h:65 * h + 65]),
                                    c(ptt[:, 512 * hh:512 * hh + 512]),
                                    start=(kt == 0), stop=(kt == 15))
                        for hh in range(2):
                            # denom row -> SBUF, broadcast to 64 partitions
                            # via ones-matmul, then full-width reciprocal
                            denr = smpool.tile([1, 512], DT, tag="denr",
                                               name=f"dn_{qb}_{p}_{hh}")
                            nc.vector.tensor_copy(denr, pvt[hh][64:65, :])
                            rb = rbps.tile([64, 512], f32, tag="rb",
                                           name=f"rb_{qb}_{p}_{hh}")
                            nc.tensor.matmul(rb, c(ones32), c(denr),
                                             start=True, stop=True)
                            rbs = smpool.tile([64, 512], f32, tag="rbs",
                                              name=f"rbs_{qb}_{p}_{hh}")
                            nc.vector.reciprocal(rbs, rb)
                            nc.vector.tensor_mul(
                                at_sb[64 * hh:64 * hh + 64, p, qsl],
                                pvt[hh][0:64, :], rbs)
                    # output projection for this query block's 4 token tiles
                    for mt in range(4 * qb, 4 * qb + 4):
                        yp = [yps.tile([128, 512], f32, tag="y",
                                       name=f"yp_{mt}_{nb}") for nb in range(2)]
                        for nb in range(2):
                            for p2 in range(2):
                                nc.tensor.matmul(
                                    yp[nb],
                                    c(at_sb[:, p2, mt * 128:(mt + 1) * 128]),
                                    c(wo_sb[:, p2, nb * 512:(nb + 1) * 512]),
                                    start=(p2 == 0), stop=(p2 == 1))
                        yo = ypool.tile([128, E], f32, tag="yo",
                                        name=f"yo_{mt}")
                        for nb in range(2):
                            nc.vector.tensor_copy(yo[:, nb * 512:(nb + 1) * 512],
                                                  yp[nb])
                        nc.gpsimd.dma_start(y_ap[mt * 128:(mt + 1) * 128, :], yo)

    nc.compile()
    return nc


def _get_program(mode):
    if mode not in _PROGRAMS:
        _PROGRAMS[mode] = _build(mode)
    return _PROGRAMS[mode]


def kernel(q, k, v, mask, Wq, bq, Wk, bk, Wv, bv, Wo, bo):
    global LAST_RESULT
    from concourse.bass_utils import run_bass_kernel_spmd

    mode = MODE
    nc = _get_program(mode)

    if mode == "bf16":
        import ml_dtypes
        cdt = ml_dtypes.bfloat16
    else:
        cdt = np.float32

    def prep(a):
        return np.ascontiguousarray(np.asarray(a).astype(cdt))

    q = np.asarray(q); k = np.asarray(k); v = np.asarray(v)
    Wq = np.asarray(Wq); Wk = np.asarray(Wk); Wv = np.asarray(Wv)
    Wo = np.asarray(Wo)
    bq = np.asarray(bq); bk = np.asarray(bk); bv = np.asarray(bv)
    bo = np.asarray(bo)

    xT = [[prep(q[b].T), prep(k[b].T), prep(v[b].T)] for b in range(B)]

    in_maps = []
    for core in range(NCORES):
        b, g = core // 4, core % 4
        r0 = g * FPC

        def wqk_layout(W):
            # lhsT tiles: [part p, ktile, m] = W.T[kt*128+p, m]
            A = W[r0:r0 + FPC, :].T.reshape(8, 128, FPC)
            return prep(A.transpose(1, 0, 2))

        WvT = Wv[r0:r0 + FPC, :].T  # (E, 256)
        Wv_aug = np.zeros((E, VW), np.float32)
        bv_aug = np.zeros((1, VW), np.float32)
        for h in range(4):
            Wv_aug[:, 65 * h:65 * h + 64] = WvT[:, 64 * h:64 * h + 64]
            bv_aug[0, 65 * h:65 * h + 64] = bv[r0 + 64 * h:r0 + 64 * h + 64]
            bv_aug[0, 65 * h + 64] = 1.0
        Wo_l = Wo[:, r0:r0 + FPC].T.reshape(2, 128, E).transpose(1, 0, 2)

        in_maps.append({
            "xq": xT[b][0], "xk": xT[b][1], "xv": xT[b][2],
            "wq": wqk_layout(Wq),
            "wk": wqk_layout(Wk),
            "wv": prep(Wv_aug.reshape(8, 128, VW).transpose(1, 0, 2)),
            "wo": prep(Wo_l),
            "bqk": np.stack([bq[r0:r0 + 128], bq[r0 + 128:r0 + FPC],
                             bk[r0:r0 + 128], bk[r0 + 128:r0 + FPC]],
                            axis=1).astype(np.float32),
            "bv": prep(bv_aug),
            "ones": np.ones((1, 128), cdt),
        })

    kwargs = {}
    if TRACE:
        kwargs = {"trace": True, "tmpdir": TRACE_DIR}
    res = run_bass_kernel_spmd(nc, in_maps, list(range(NCORES)), **kwargs)
    LAST_RESULT = res

    y = np.zeros((B, S, E), np.float32)
    for core in range(NCORES):
        y[core // 4] += res.results[core]["y"]
    y += bo.astype(np.float32)
    return y


# revision 21
# speedup vs baseline: 2.4731x; 1.1850x over previous
"""Multi-head attention (B=2, S=2048, E=1024, H=16) on 8 Trainium2 cores.

Sharding: core c -> (batch b = c//4, head-group g = c%4 of 4 heads).
Each core computes Q/K/V projections for its 4 heads (256 features),
full attention for those heads, and a partial output projection
(256 rows of Wo). Host sums the 4 partials per batch element and adds bo.

On-chip layouts (per core):
  qt/kt: (128 feat-part, pair, 2048 tok)  transposed proj outputs; the
         128 partitions hold two heads (64+64) per pair index.
  v:     (128 tok-part, 16 tok-tiles, 4*65): per head 64 dims plus a
         "ones" column produced by an augmented V projection (extra
         output feature with zero weights and bias 1.0); P @ V_aug then
         also yields the softmax denominator row for free.
  scores are computed transposed (key-pos on partitions, query on free)
  so exp runs on ACT along the free dim and P tiles feed P@V directly as
  the moving operand; no transposes anywhere in the pipeline.

All weight matrices are re-laid-out on the host so every DMA is a
contiguous per-partition run.
"""

import numpy as np

B, S, E, H = 2, 2048, 1024, 16
D = 64
NCORES = 8
FPC = 256  # features (head dims) per core = 4 heads
VW = 4 * 65  # V-projection output width incl. ones columns

# 'f32' (exact, 4 cyc/row), 'f32r' (fp32 storage, rounded fast PE mode), 'bf16'
MODE = "bf16"

_PROGRAMS = {}
LAST_RESULT = None
TRACE = False
TRACE_DIR = None


def _build(mode):
    import concourse.tile as tile
    from concourse import bacc, mybir

    f32 = mybir.dt.float32
    if mode == "bf16":
        DT = mybir.dt.bfloat16
    elif mode == "f32r":
        DT = mybir.dt.float32r
    else:
        DT = f32
    # moving-dim block: matmul fp32 PSUM output caps one bank = 512 floats
    NW = 512
    NNB = S // NW

    nc = bacc.Bacc("TRN2", target_bir_lowering=False, debug=False,
                   num_devices=NCORES)

    xq_ap = nc.dram_tensor("xq", [E, S], DT, kind="ExternalInput").ap()
    xk_ap = nc.dram_tensor("xk", [E, S], DT, kind="ExternalInput").ap()
    xv_ap = nc.dram_tensor("xv", [E, S], DT, kind="ExternalInput").ap()
    wq_ap = nc.dram_tensor("wq", [128, 8, FPC], DT, kind="ExternalInput").ap()
    wk_ap = nc.dram_tensor("wk", [128, 8, FPC], DT, kind="ExternalInput").ap()
    wv_ap = nc.dram_tensor("wv", [128, 8, VW], DT, kind="ExternalInput").ap()
    wo_ap = nc.dram_tensor("wo", [128, 2, E], DT, kind="ExternalInput").ap()
    bqk_ap = nc.dram_tensor("bqk", [128, 4], f32, kind="ExternalInput").ap()
    bv_ap = nc.dram_tensor("bv", [1, VW], DT, kind="ExternalInput").ap()
    ones_ap = nc.dram_tensor("ones", [1, 128], DT, kind="ExternalInput").ap()
    y_ap = nc.dram_tensor("y", [S, E], f32, kind="ExternalOutput").ap()

    Exp = mybir.ActivationFunctionType.Exp

    with tile.TileContext(nc) as tc:
        with tc.tile_pool(name="persist", bufs=1) as persist:
            wq_sb = persist.tile([128, 8, FPC], DT, name="wq_sb")
            wk_sb = persist.tile([128, 8, FPC], DT, name="wk_sb")
            wv_sb = persist.tile([128, 8, VW], DT, name="wv_sb")
            wo_sb = persist.tile([128, 2, E], DT, name="wo_sb")
            bqk_sb = persist.tile([128, 4], f32, name="bqk_sb")
            bv_sb = persist.tile([1, VW], DT, name="bv_sb")
            ones_sb = persist.tile([1, 128], DT, name="ones_sb")
            # weights/constants on the GpSimd DGE queue, x loads on Sync's:
            # two parallel streams so the first matmul isn't gated on the
            # whole preload.
            nc.gpsimd.dma_start(wq_sb, wq_ap)
            nc.gpsimd.dma_start(wk_sb, wk_ap)
            nc.gpsimd.dma_start(wv_sb, wv_ap)
            nc.gpsimd.dma_start(wo_sb, wo_ap)
            nc.gpsimd.dma_start(bqk_sb, bqk_ap)
            nc.gpsimd.dma_start(bv_sb, bv_ap)
            nc.gpsimd.dma_start(ones_sb, ones_ap)
            ones32 = ones_sb[:, 0:64]

            qt_sb = persist.tile([128, 2, S], DT, name="qt_sb")
            kt_sb = persist.tile([128, 2, S], DT, name="kt_sb")
            v_sb = persist.tile([128, 16, VW], DT, name="v_sb")
            at_sb = persist.tile([128, 2, S], DT, name="at_sb")

            with tc.tile_pool(name="xs", bufs=3) as xpool:
                # ---- Q and K projections: out = (feat-part, tok) ----
                with tc.tile_pool(name="pjqk", bufs=2 * NNB,
                                  space="PSUM") as pjqk:
                    for which, xap, w_sb, out_sb, bcol in (
                        (0, xq_ap, wq_sb, qt_sb, 0),
                        (1, xk_ap, wk_sb, kt_sb, 2),
                    ):
                        ps = {}
                        for mt in range(2):
                            for nb in range(NNB):
                                ps[(mt, nb)] = pjqk.tile(
                                    [128, NW], f32, tag="pj",
                                    name=f"pjq_{which}_{mt}_{nb}")
                        for kt in range(8):
                            xt = xpool.tile([128, S], DT, tag="x",
                                            name=f"x_{which}_{kt}")
                            nc.sync.dma_start(xt,
                                              xap[kt * 128:(kt + 1) * 128, :])
                            for mt in range(2):
                                for nb in range(NNB):
                                    nc.tensor.matmul(
                                        ps[(mt, nb)],
                                        w_sb[:, kt, mt * 128:(mt + 1) * 128],
                                        xt[:, nb * NW:(nb + 1) * NW],
                                        start=(kt == 0), stop=(kt == 7))
                        for mt in range(2):
                            for nb in range(NNB):
                                nc.vector.tensor_scalar_add(
                                    out_sb[:, mt, nb * NW:(nb + 1) * NW],
                                    ps[(mt, nb)],
                                    bqk_sb[:, bcol + mt:bcol + mt + 1])

                # ---- V projection: out = (tok-part, 4*65 feat) ----
                # bias row via K=1 ones-matmul; the augmented columns carry
                # zero weights + bias 1.0 -> ones columns for the denominator
                with tc.tile_pool(name="pjv", bufs=8, space="PSUM") as pjv:
                    for half in range(2):
                        psv = [pjv.tile([128, VW], f32, tag="pjv",
                                        name=f"pjv_{half}_{i}")
                               for i in range(8)]
                        for i in range(8):
                            nc.tensor.matmul(psv[i], ones_sb, bv_sb,
                                             start=True, stop=False)
                        for kt in range(8):
                            xt = xpool.tile([128, 1024], DT, tag="x",
                                            name=f"xv_{half}_{kt}")
                            nc.sync.dma_start(
                                xt, xv_ap[kt * 128:(kt + 1) * 128,
                                          half * 1024:(half + 1) * 1024])
                            for i in range(8):
                                nc.tensor.matmul(
                                    psv[i],
                                    xt[:, i * 128:(i + 1) * 128],
                                    wv_sb[:, kt, :],
                                    start=False, stop=(kt == 7))
                        for i in range(8):
                            tt = half * 8 + i
                            nc.vector.tensor_copy(v_sb[:, tt, :], psv[i])

            # ---- attention + output projection ----
            # psum: 3 "sc" slots of 2 banks (pair-merged scores; also
            # borrowed by the denominator broadcast and output projection)
            # + 2 single-bank pv slots = 8 banks.
            with tc.tile_pool(name="pt", bufs=8) as ptpool, \
                 tc.tile_pool(name="sm", bufs=2) as smpool, \
                 tc.tile_pool(name="ysb", bufs=3) as ypool, \
                 tc.tile_pool(name="scps", bufs=3, space="PSUM") as scps, \
                 tc.tile_pool(name="pvps", bufs=2, space="PSUM") as pvps:
                for qb in range(NNB):
                    qsl = slice(qb * NW, (qb + 1) * NW)
                    for p in range(2):
                        pvt = [pvps.tile([65, NW], f32, tag="pv",
                                         name=f"pv_{qb}_{p}_{hh}")
                               for hh in range(2)]
                        for kt in range(16):
                            # both heads' transposed scores in one 2-bank
                            # tile; a single exp covers the pair
                            s_ = scps.tile([128, 2 * NW], f32, tag="sc",
                                           name=f"sc_{qb}_{p}_{kt}")
                            for hh in range(2):
                                nc.tensor.matmul(
                                    s_[:, NW * hh:NW * hh + NW],
                                    kt_sb[64 * hh:64 * hh + 64, p,
                                          kt * 128:(kt + 1) * 128],
                                    qt_sb[64 * hh:64 * hh + 64, p, qsl],
                                    start=True, stop=True)
                            ptt = ptpool.tile([128, 2 * NW], DT, tag="pt",
                                              name=f"pt_{qb}_{p}_{kt}")
                            nc.scalar.activation(ptt, s_, Exp, scale=0.125)
                            for hh in range(2):
                                h = 2 * p + hh
                                nc.tensor.matmul(
                                    pvt[hh],
                                    v_sb[:, kt, 65 * h:65 * h + 65],
                                    ptt[:, NW * hh:NW * hh + NW],
                                    start=(kt == 0), stop=(kt == 15))
                        for hh in range(2):
                            # denom row -> SBUF, broadcast to 64 partitions
                            # via ones-matmul, then full-width reciprocal
                            denr = smpool.tile([1, NW], DT, tag="denr",
                                               name=f"dn_{qb}_{p}_{hh}")
                            nc.vector.tensor_copy(denr, pvt[hh][64:65, :])
                            rb = scps.tile([64, NW], f32, tag="sc",
                                           name=f"rb_{qb}_{p}_{hh}")
                            nc.tensor.matmul(rb, ones32, denr,
                                             start=True, stop=True)
                            rbs = smpool.tile([64, NW], f32, tag="rbs",
                                              name=f"rbs_{qb}_{p}_{hh}")
                            nc.vector.reciprocal(rbs, rb)
                            nc.vector.tensor_mul(
                                at_sb[64 * hh:64 * hh + 64, p, qsl],
                                pvt[hh][0:64, :], rbs)
                    # output projection for this query block's token tiles
                    for mt in range(4 * qb, 4 * qb + 4):
                        yp = scps.tile([128, 2 * NW], f32, tag="sc",
                                       name=f"yp_{mt}")
                        for nb in range(2):
                            for p2 in range(2):
                                nc.tensor.matmul(
                                    yp[:, nb * NW:(nb + 1) * NW],
                                    at_sb[:, p2, mt * 128:(mt + 1) * 128],
                                    wo_sb[:, p2, nb * NW:(nb + 1) * NW],
                                    start=(p2 == 0), stop=(p2 == 1))
                        yo = ypool.tile([128, E], f32, tag="yo",
                                        name=f"yo_{mt}")
                        nc.vector.tensor_copy(yo, yp)
                        nc.gpsimd.dma_start(y_ap[mt * 128:(mt + 1) * 128, :],
                                            yo)

    nc.compile()
    return nc


def _get_program(mode):
    if mode not in _PROGRAMS:
        _PROGRAMS[mode] = _build(mode)
    return _PROGRAMS[mode]


def kernel(q, k, v, mask, Wq, bq, Wk, bk, Wv, bv, Wo, bo):
    global LAST_RESULT
    from concourse.bass_utils import run_bass_kernel_spmd

    mode = MODE
    nc = _get_program(mode)

    if mode == "bf16":
        import ml_dtypes
        cdt = ml_dtypes.bfloat16
    else:
        cdt = np.float32

    def prep(a):
        return np.ascontiguousarray(np.asarray(a).astype(cdt))

    q = np.asarray(q); k = np.asarray(k); v = np.asarray(v)
    Wq = np.asarray(Wq); Wk = np.asarray(Wk); Wv = np.asarray(Wv)
    Wo = np.asarray(Wo)
    bq = np.asarray(bq); bk = np.asarray(bk); bv = np.asarray(bv)
    bo = np.asarray(bo)

    xT = [[prep(q[b].T), prep(k[b].T), prep(v[b].T)] for b in range(B)]

    in_maps = []
    for core in range(NCORES):
        b, g = core // 4, core % 4
        r0 = g * FPC

        def wqk_layout(W):
            # lhsT tiles: [part p, ktile, m] = W.T[kt*128+p, m]
            A = W[r0:r0 + FPC, :].T.reshape(8, 128, FPC)
            return prep(A.transpose(1, 0, 2))

        WvT = Wv[r0:r0 + FPC, :].T  # (E, 256)
        Wv_aug = np.zeros((E, VW), np.float32)
        bv_aug = np.zeros((1, VW), np.float32)
        for h in range(4):
            Wv_aug[:, 65 * h:65 * h + 64] = WvT[:, 64 * h:64 * h + 64]
            bv_aug[0, 65 * h:65 * h + 64] = bv[r0 + 64 * h:r0 + 64 * h + 64]
            bv_aug[0, 65 * h + 64] = 1.0
        Wo_l = Wo[:, r0:r0 + FPC].T.reshape(2, 128, E).transpose(1, 0, 2)

        in_maps.append({
            "xq": xT[b][0], "xk": xT[b][1], "xv": xT[b][2],
            "wq": wqk_layout(Wq),
            "wk": wqk_layout(Wk),
            "wv": prep(Wv_aug.reshape(8, 128, VW).transpose(1, 0, 2)),
            "wo": prep(Wo_l),
            "bqk": np.stack([bq[r0:r0 + 128], bq[r0 + 128:r0 + FPC],
                             bk[r0:r0 + 128], bk[r0 + 128:r0 + FPC]],
                            axis=1).astype(np.float32),
            "bv": prep(bv_aug),
            "ones": np.ones((1, 128), cdt),
        })

    kwargs = {}
    if TRACE:
        kwargs = {"trace": True, "tmpdir": TRACE_DIR}
    res = run_bass_kernel_spmd(nc, in_maps, list(range(NCORES)), **kwargs)
    LAST_RESULT = res

    y = np.zeros((B, S, E), np.float32)
    for core in range(NCORES):
        y[core // 4] += res.results[core]["y"]
    y += bo.astype(np.float32)
    return y
